# revision 1
# baseline (speedup 1.0000x reference)
"""Trainium2 Bass kernel: adaptive-input softmax ('softmax' mode), 8 NeuronCores.

Strategy: vocab tensor-parallel. Each core owns a 1/8 slice of the head token
columns (2500 of 20000), tail0 columns (2500 of 20000) and tail1 columns
(1283 of ceil(10257/8)*8, zero-padded), computes partition-local logits ->
exp, and the per-row softmax denominators are completed with a small
cross-core AllGather of per-row exp-sums (overlapped with compute).
The 2 cluster logits are computed replicated on every core and folded into
the head sums (scaled 1/8) before the AllGather.

Precision plan: the head matmul (99.99% of output probability mass) runs in
bf16. The tail projections and the tail0 logit matmul run in fp8e4m3 with
DoubleRow perf mode (2 k-tiles merged per instruction at 0.5 cycles/row =
4x fewer PE cycles); the tails carry ~1e-4 of the probability mass so their
~1% internal error is invisible in the rel-l2 metric. Scale management:
host stores p0*16, p1*16, w0*8 in fp8; the kernel stores h0/8 in fp8 so
h0/8 @ (8*w0) reproduces h0@w0 exactly. Output stores in bf16 (upcast on
host).
"""
import numpy as np
import ml_dtypes
from contextlib import ExitStack

import concourse.bass as bass
import concourse.tile as tile
from concourse import bacc, mybir
from concourse.bass_utils import run_bass_kernel_spmd
from concourse.masks import make_identity

N_CORES = 8
D = 1024
KT = D // 128                      # contraction k-tiles over D
B0 = 20000                         # head token columns
HEAD_SLICE = B0 // N_CORES         # 2500 per core
T0_ALL = 20000
T0_SLICE = T0_ALL // N_CORES       # 2500 per core
T1_ALL = 10257
T1_SLICE = -(-T1_ALL // N_CORES)   # 1283 per core (global pad to 10264)
T1_PADDED = T1_SLICE * N_CORES
HEAD_COLS = HEAD_SLICE + 2         # + 2 replicated cluster columns
OUT_COLS = HEAD_SLICE + T0_SLICE + T1_SLICE   # 6283 per-core output columns
P0 = 256                           # tail0 projection dim
P1 = 64                            # tail1 projection dim
V = B0 + T0_ALL + T1_ALL           # 50257
BLK = 3                            # m-tiles per AllGather block
SW = 16.0                          # head-weight fp8 pre-scale

BF16 = mybir.dt.bfloat16
F32 = mybir.dt.float32
F8 = mybir.dt.float8e4
DR = mybir.MatmulPerfMode.DoubleRow
EXP = mybir.ActivationFunctionType.Exp
COPY = mybir.ActivationFunctionType.Copy
ADD = mybir.AluOpType.add
AX = mybir.AxisListType.X


def _tiles(total, step=512):
    out, off = [], 0
    while off < total:
        w = min(step, total - off)
        out.append((off, w))
        off += w
    return out


def build(rows):
    assert rows % 256 == 0
    m_tiles = rows // 128

    nc = bacc.Bacc("TRN2", target_bir_lowering=False, debug=False,
                   num_devices=N_CORES)
    xT_ext = nc.declare_dram_parameter("xT", [D, rows], BF16, isOutput=False)
    x8_ext = nc.declare_dram_parameter("x8", [D, rows], F8, isOutput=False)
    wh_ext = nc.declare_dram_parameter("wh", [D, HEAD_COLS], BF16, isOutput=False)
    p0_ext = nc.declare_dram_parameter("p0", [D, P0], F8, isOutput=False)
    w0_ext = nc.declare_dram_parameter("w0", [P0, T0_SLICE], F8, isOutput=False)
    p1_ext = nc.declare_dram_parameter("p1", [D, P1], F8, isOutput=False)
    w1_ext = nc.declare_dram_parameter("w1", [P1, T1_SLICE], BF16, isOutput=False)
    npad_ext = nc.declare_dram_parameter("negpad", [128, 1], F32, isOutput=False)
    out_ext = nc.declare_dram_parameter("out", [rows, OUT_COLS], BF16, isOutput=True)

    head_tiles = _tiles(HEAD_COLS)        # last tile contains 2 cluster cols
    t0_tiles = _tiles(T0_SLICE)
    t1_tiles = _tiles(T1_SLICE)
    nh, n0, n1 = len(head_tiles), len(t0_tiles), len(t1_tiles)
    proj_tiles = _tiles(rows, 512)
    out_chunks = _tiles(OUT_COLS, -(-OUT_COLS // 3))   # 3 even-ish store chunks
    CH = out_chunks[0][1]

    with ExitStack() as ctx:
        tc = ctx.enter_context(tile.TileContext(nc))
        const = ctx.enter_context(tc.tile_pool(name="const", bufs=1))
        psum_pool = ctx.enter_context(tc.tile_pool(name="psum", bufs=6, space="PSUM"))
        psum_a = ctx.enter_context(tc.tile_pool(name="psum_a", bufs=1, space="PSUM"))
        psum_g = ctx.enter_context(tc.tile_pool(name="psum_g", bufs=1, space="PSUM"))
        exp_pool = ctx.enter_context(tc.tile_pool(name="exppool", bufs=6))
        outp = ctx.enter_context(tc.tile_pool(name="outp", bufs=3))
        small = ctx.enter_context(tc.tile_pool(name="small", bufs=7))
        dram = ctx.enter_context(tc.tile_pool(name="dram", bufs=3, space="DRAM"))

        # ---------- resident inputs ----------
        xT_sb = const.tile([128, KT, rows], BF16, name="xT_sb")
        x8_sb = const.tile([128, KT, rows], F8, name="x8_sb")
        wh_sb = const.tile([128, KT, HEAD_COLS], BF16, name="wh_sb")
        p0_sb = const.tile([128, KT, P0], F8, name="p0_sb")
        p1_sb = const.tile([128, KT, P1], F8, name="p1_sb")
        w0_sb = const.tile([128, P0 // 128, T0_SLICE], F8, name="w0_sb")
        w1_sb = const.tile([P1, T1_SLICE], BF16, name="w1_sb")
        npad_sb = const.tile([128, 1], F32, name="npad_sb")

        # warm-up AllGather first, with no dependencies at all, so the
        # ~50-70us first-collective init burns on the CC engine starting at
        # t=0, fully inside the load phase
        warm_sb = small.tile([1, 128], F32, name="warm_sb")
        nc.gpsimd.memset(warm_sb[:, :], 0.0)
        warm_in = dram.tile([1, 128], F32, name="warm_in")
        warm_out = dram.tile([N_CORES, 128], F32, name="warm_out",
                             addr_space="Shared")
        nc.gpsimd.dma_start(out=warm_in[:, :], in_=warm_sb[:, :])
        nc.gpsimd.collective_compute(
            "AllGather", mybir.AluOpType.bypass,
            replica_groups=[list(range(N_CORES))],
            ins=[warm_in.opt()], outs=[warm_out.opt()],
        )
        ident = const.tile([128, 128], F32, name="ident")
        make_identity(nc, ident)

        # loads, spread over the 3 DMA-capable queues in consumption order.
        def _wh_chunk(q, ti, sb, ext):
            off, w = head_tiles[ti]
            for k in range(KT):
                q.dma_start(out=sb[:, k, off:off + w],
                            in_=ext[k * 128:(k + 1) * 128, off:off + w])

        def _x_chunk(q, rt, sb, ext):
            roff, rw = proj_tiles[rt]
            for k in range(KT):
                q.dma_start(out=sb[:, k, roff:roff + rw],
                            in_=ext[k * 128:(k + 1) * 128, roff:roff + rw])

        # sync q: xT row-chunk 0 (unblocks m-tile 0), wh ct2, then the rest
        # of xT/x8 interleaved by row-chunk (x8 rt is needed by proj(rt))
        _x_chunk(nc.sync, 0, xT_sb, xT_ext)
        _wh_chunk(nc.sync, 2, wh_sb, wh_ext)
        _x_chunk(nc.sync, 0, x8_sb, x8_ext)
        for rt in range(1, len(proj_tiles)):
            _x_chunk(nc.sync, rt, xT_sb, xT_ext)
            _x_chunk(nc.sync, rt, x8_sb, x8_ext)
        # scalar q: wh ct0, ct1, then tail weights
        _wh_chunk(nc.scalar, 0, wh_sb, wh_ext)
        _wh_chunk(nc.scalar, 1, wh_sb, wh_ext)
        for k in range(P0 // 128):
            nc.scalar.dma_start(out=w0_sb[:, k, :], in_=w0_ext[k * 128:(k + 1) * 128, :])
        nc.scalar.dma_start(out=w1_sb[:, :], in_=w1_ext[:, :])
        nc.scalar.dma_start(out=npad_sb[:], in_=npad_ext[:])
        # gpsimd q: p0, p1 (needed after m0's head), then wh ct3, ct4
        for k in range(KT):
            nc.gpsimd.dma_start(out=p0_sb[:, k, :], in_=p0_ext[k * 128:(k + 1) * 128, :])
        for k in range(KT):
            nc.gpsimd.dma_start(out=p1_sb[:, k, :], in_=p1_ext[k * 128:(k + 1) * 128, :])
        for ti in range(3, nh):
            _wh_chunk(nc.gpsimd, ti, wh_sb, wh_ext)

        # ---------- tail hidden projections (fp8 DoubleRow) ----------
        # h0T8 holds h0/8 in fp8: psum = x8 @ (16 p0) = 16 h0, Act scale
        # 1/128 -> h0/8.  h1T holds h1 in bf16 (psum = 16 h1, scale 1/16).
        h0T8_sb = const.tile([128, P0 // 128, rows], F8, name="h0T8_sb")
        h1T_sb = const.tile([P1, rows], BF16, name="h1T_sb")

        def emit_proj(rt):
            roff, rw = proj_tiles[rt]
            for mp in range(P0 // 128):
                ps = psum_pool.tile([128, 512], F32, name="ps")
                for kp in range(KT // 2):
                    nc.tensor.matmul(ps[:, :rw],
                                     lhsT=p0_sb[:, 2 * kp:2 * kp + 2, mp * 128:(mp + 1) * 128],
                                     rhs=x8_sb[:, 2 * kp:2 * kp + 2, roff:roff + rw],
                                     start=(kp == 0), stop=(kp == KT // 2 - 1),
                                     perf_mode=DR)
                nc.scalar.activation(h0T8_sb[:, mp, roff:roff + rw], ps[:, :rw],
                                     COPY, scale=1.0 / 128.0)
            ps = psum_pool.tile([128, 512], F32, name="ps")
            for kp in range(KT // 2):
                nc.tensor.matmul(ps[:P1, :rw],
                                 lhsT=p1_sb[:, 2 * kp:2 * kp + 2, :],
                                 rhs=x8_sb[:, 2 * kp:2 * kp + 2, roff:roff + rw],
                                 start=(kp == 0), stop=(kp == KT // 2 - 1),
                                 perf_mode=DR)
            nc.scalar.activation(h1T_sb[:, roff:roff + rw], ps[:P1, :rw],
                                 COPY, scale=1.0 / 16.0)

        # ---------- per m-tile compute (head -> proj -> tails) ----------
        n_rt = len(proj_tiles)

        def emit_compute(m, j, sums_blk, per_m):
            r0 = m * 128
            exph = exp_pool.tile([128, HEAD_SLICE], BF16, name="exph")
            expt0 = exp_pool.tile([128, T0_SLICE], BF16, name="expt0")
            expt1 = exp_pool.tile([128, T1_SLICE], BF16, name="expt1")
            partials = small.tile([128, nh + n0 + n1], F32, name="partials")
            cexp = small.tile([128, 2], F32, name="cexp")
            j3 = j * 3

            pcol = 0
            for ti, (off, w) in enumerate(head_tiles):
                ps = psum_pool.tile([128, 512], F32, name="ps")
                for k in range(KT):
                    nc.tensor.matmul(ps[:, :w], lhsT=xT_sb[:, k, r0:r0 + 128],
                                     rhs=wh_sb[:, k, off:off + w],
                                     start=(k == 0), stop=(k == KT - 1))
                if ti == nh - 1:
                    wt = w - 2   # exclude the 2 cluster cols from sum/output
                    nc.scalar.activation(exph[:, off:off + wt], ps[:, :wt], EXP,
                                         accum_out=partials[:, pcol:pcol + 1])
                    nc.scalar.activation(cexp[:, :], ps[:, wt:w], EXP)
                else:
                    nc.scalar.activation(exph[:, off:off + w], ps[:, :w], EXP,
                                         accum_out=partials[:, pcol:pcol + 1])
                pcol += 1
            # head-sum with the replicated cluster exps folded in (scaled by
            # 1/8 so the 8-way AllGather add reconstructs them exactly once);
            # emitted here, off the block-end critical path
            hraw = small.tile([128, 1], F32, name="hraw")
            cs = small.tile([128, 1], F32, name="cs")
            cs8 = small.tile([128, 1], F32, name="cs8")
            nc.vector.tensor_reduce(out=hraw[:], in_=partials[:, 0:nh],
                                    axis=AX, op=ADD)
            nc.vector.tensor_reduce(out=cs[:], in_=cexp[:, 0:2], axis=AX, op=ADD)
            nc.vector.tensor_scalar_mul(cs8[:], cs[:], 1.0 / N_CORES)
            nc.vector.tensor_add(sums_blk[:, j3:j3 + 1], hraw[:], cs8[:])

            # interleave the projection for row-tile m here: m-tile m only
            # needs proj row-tile m//4, emitted for m<4 keeps the PE dense
            if m < n_rt:
                emit_proj(m)
            for (off, w) in t0_tiles:
                ps = psum_pool.tile([128, 512], F32, name="ps")
                nc.tensor.matmul(ps[:, :w], lhsT=h0T8_sb[:, :, r0:r0 + 128],
                                 rhs=w0_sb[:, :, off:off + w],
                                 start=True, stop=True, perf_mode=DR)
                nc.scalar.activation(expt0[:, off:off + w], ps[:, :w], EXP,
                                     accum_out=partials[:, pcol:pcol + 1])
                pcol += 1
            for (off, w) in t1_tiles:
                ps = psum_pool.tile([128, 512], F32, name="ps")
                nc.tensor.matmul(ps[:, :w], lhsT=h1T_sb[:, r0:r0 + 128],
                                 rhs=w1_sb[:, off:off + w],
                                 start=True, stop=True)
                nc.scalar.activation(expt1[:, off:off + w], ps[:, :w], EXP,
                                     accum_out=partials[:, pcol:pcol + 1])
                pcol += 1

            t1raw = small.tile([128, 1], F32, name="t1raw")
            nc.vector.tensor_reduce(out=sums_blk[:, j3 + 1:j3 + 2],
                                    in_=partials[:, nh:nh + n0], axis=AX, op=ADD)
            nc.vector.tensor_reduce(out=t1raw[:], in_=partials[:, nh + n0:nh + n0 + n1],
                                    axis=AX, op=ADD)
            nc.vector.tensor_add(sums_blk[:, j3 + 2:j3 + 3], t1raw[:], npad_sb[:])
            per_m[m] = (exph, expt0, expt1, cexp)

        def emit_comm(blk, sums_blk):
            # One AllGather per block of up to BLK m-tiles. Sums travel
            # transposed ([3*bs, 128] rows) so every DMA burst is 512B.
            bs = len(blk)
            sc = bs * 3
            psT = psum_a.tile([BLK * 3, 128], F32, name="psT")
            nc.tensor.transpose(psT[:sc, :], sums_blk[:, :sc], ident[:, :])
            sumsT = small.tile([BLK * 3, 128], F32, name="sumsT", bufs=2)
            # on ScalarE: keeps the AllGather issue path off the DVE queue
            nc.scalar.copy(sumsT[:sc, :], psT[:sc, :])
            cc_in = dram.tile([sc, 128], F32, name=f"cc_in{bs}")
            cc_out = dram.tile([N_CORES * sc, 128], F32, name=f"cc_out{bs}",
                               addr_space="Shared")
            nc.gpsimd.dma_start(out=cc_in[:, :], in_=sumsT[:sc, :])
            nc.gpsimd.collective_compute(
                "AllGather", mybir.AluOpType.bypass,
                replica_groups=[list(range(N_CORES))],
                ins=[cc_in.opt()], outs=[cc_out.opt()],
            )
            return cc_out

        # stores ride the sync queue: a store's semaphore wait (on the
        # DVE scale-mul producing its staging tile) would otherwise block
        # subsequent exp work on the Act queue, stalling PSUM drain -> PE.
        # (gpsimd is used as a second store queue only in the final return,
        # when no further collective needs its queue.)
        store_q = [nc.sync, nc.gpsimd]
        store_n = [0]

        def emit_return(blk, per_m, cc_out, final=False):
            bs = len(blk)
            sc = bs * 3
            gsall = small.tile([N_CORES * sc, 128], F32, name=f"gsall{bs}", bufs=2)
            nc.gpsimd.dma_start(out=gsall[:, :], in_=cc_out[:, :])
            gstp = psum_g.tile([128, N_CORES * BLK * 3], F32, name="gstp")
            nc.tensor.transpose(gstp[:, :N_CORES * sc], gsall[:, :],
                                ident[:N_CORES * sc, :N_CORES * sc])
            # free layout of gstp: (rank r, m-index jj, col c) -> r*sc + jj*3 + c
            gst_v = gstp[:, :N_CORES * sc].rearrange("p (r s) -> p s r", r=N_CORES)
            for jj, m in enumerate(blk):
                exph, expt0, expt1, cexp = per_m[m]
                r0 = m * 128
                # gs: [head_den, t0_den, t1_den] (cluster exps already folded
                # into head_den on the compute side); read directly from PSUM
                gs = small.tile([128, 3], F32, name="gs")
                nc.vector.tensor_reduce(out=gs[:, :],
                                        in_=gst_v[:, jj * 3:(jj + 1) * 3, :],
                                        axis=AX, op=ADD)
                rec = small.tile([128, 3], F32, name="rec")
                nc.vector.reciprocal(rec[:, :], gs[:, :])
                u = small.tile([128, 2], F32, name="u")
                nc.vector.tensor_scalar_mul(u[:, :], cexp[:, :], rec[:, 0:1])
                ts = small.tile([128, 2], F32, name="ts")
                nc.vector.tensor_mul(ts[:, :], u[:, :], rec[:, 1:3])

                # scale + store in wide staging tiles (big contiguous DMA bursts)
                sections = [(0, HEAD_SLICE, exph, rec[:, 0:1]),
                            (HEAD_SLICE, T0_SLICE, expt0, ts[:, 0:1]),
                            (HEAD_SLICE + T0_SLICE, T1_SLICE, expt1, ts[:, 1:2])]
                for ci, (soff, sw) in enumerate(out_chunks):
                    # in the final return there is no more exp/AG work, so
                    # give each m-tile its own mul engine (Act vs DVE) and
                    # store queue (gpsimd vs sync): two parallel drains
                    on_act = final and jj % 2 == 0 and bs > 1
                    sq = store_q[1] if (final and jj % 2 == 0 and bs > 1) else store_q[0]
                    ot = outp.tile([128, CH], BF16, name="ot")
                    for (base, slen, exp_t, scale_ap) in sections:
                        lo = max(soff, base)
                        hi = min(soff + sw, base + slen)
                        if lo >= hi:
                            continue
                        if on_act:
                            nc.scalar.activation(
                                ot[:, lo - soff:hi - soff],
                                exp_t[:, lo - base:hi - base], COPY,
                                scale=scale_ap)
                        else:
                            nc.vector.tensor_scalar_mul(
                                ot[:, lo - soff:hi - soff],
                                exp_t[:, lo - base:hi - base], scale_ap)
                    sq.dma_start(
                        out=out_ext[r0:r0 + 128, soff:soff + sw], in_=ot[:, :sw])
                    store_n[0] += 1

        # blocks of BLK m-tiles; the final two blocks are 2 m-tiles each so
        # the last AllGather posts right at the end of compute with the CC
        # engine already drained (one ~20us AG exposed, short return after)
        blocks = []
        rem = 0
        while m_tiles - rem >= BLK + 4 or m_tiles - rem == BLK:
            blocks.append(list(range(rem, rem + BLK)))
            rem += BLK
        while rem < m_tiles:
            take = min(2, m_tiles - rem)
            blocks.append(list(range(rem, rem + take)))
            rem += take

        # Software-pipelined: block b's AllGather is issued right after its
        # compute; block b-1's return (gather fetch, transpose, normalize,
        # store) is emitted after the SECOND m-tile of block b+1, giving the
        # AllGather ~1.6 blocks (~70us) of slack before the in-order PE hits
        # the gather-transpose that waits on it.
        pending = None
        for blk in blocks:
            per_m = {}
            sums_blk = small.tile([128, BLK * 3], F32, name="sums_blk")
            for j, m in enumerate(blk):
                emit_compute(m, j, sums_blk, per_m)
                if j == 1 and pending is not None:
                    emit_return(*pending)
                    pending = None
            cc_out = emit_comm(blk, sums_blk)
            if pending is not None:
                emit_return(*pending)
                pending = None
            pending = (blk, per_m, cc_out)
        emit_return(*pending, final=True)

    nc.compile()
    return nc


def _shard_inputs(x2d, head_weight, tail_proj_0, tail_w_0, tail_proj_1, tail_w_1):
    bf = ml_dtypes.bfloat16
    f8 = ml_dtypes.float8_e4m3fn
    xT = np.ascontiguousarray(x2d.T)
    cluster = head_weight[:, B0:B0 + 2]
    w1p = np.zeros((P1, T1_PADDED), np.float32)
    w1p[:, :T1_ALL] = tail_w_1
    xTb = xT.astype(bf)
    xT8 = xT.astype(f8)
    p08 = np.ascontiguousarray(tail_proj_0 * 16.0).astype(f8)
    p18 = np.ascontiguousarray(tail_proj_1 * 16.0).astype(f8)
    in_maps = []
    for c in range(N_CORES):
        wh = np.concatenate(
            [head_weight[:, c * HEAD_SLICE:(c + 1) * HEAD_SLICE], cluster], axis=1)
        npad = -float(T1_PADDED - T1_ALL) if c == N_CORES - 1 else 0.0
        in_maps.append({
            "xT": xTb,
            "x8": xT8,
            "wh": np.ascontiguousarray(wh).astype(bf),
            "p0": p08,
            "w0": np.ascontiguousarray(
                tail_w_0[:, c * T0_SLICE:(c + 1) * T0_SLICE] * 8.0).astype(f8),
            "p1": p18,
            "w1": np.ascontiguousarray(
                w1p[:, c * T1_SLICE:(c + 1) * T1_SLICE]).astype(bf),
            "negpad": np.full((128, 1), npad, np.float32),
        })
    return in_maps


def _assemble(outs, rows):
    full = np.empty((rows, V), np.float32)
    for c in range(N_CORES):
        o = np.asarray(outs[c]).astype(np.float32)
        full[:, c * HEAD_SLICE:(c + 1) * HEAD_SLICE] = o[:, :HEAD_SLICE]
        full[:, B0 + c * T0_SLICE:B0 + (c + 1) * T0_SLICE] = \
            o[:, HEAD_SLICE:HEAD_SLICE + T0_SLICE]
        lo = c * T1_SLICE
        hi = min((c + 1) * T1_SLICE, T1_ALL)
        full[:, B0 + T0_ALL + lo:B0 + T0_ALL + hi] = \
            o[:, HEAD_SLICE + T0_SLICE:HEAD_SLICE + T0_SLICE + (hi - lo)]
    return full


RUN_KWARGS = {}      # test harness may set e.g. {"trace": True}
LAST_RESULT = None   # test harness reads exec_time_ns / profile from here


def kernel(x, head_weight, tail_proj_0, tail_w_0, tail_proj_1, tail_w_1):
    global LAST_RESULT
    x = np.asarray(x, dtype=np.float32)
    n, t, d = x.shape
    rows = n * t
    nc = build(rows)
    in_maps = _shard_inputs(
        x.reshape(rows, d),
        np.asarray(head_weight, dtype=np.float32),
        np.asarray(tail_proj_0, dtype=np.float32),
        np.asarray(tail_w_0, dtype=np.float32),
        np.asarray(tail_proj_1, dtype=np.float32),
        np.asarray(tail_w_1, dtype=np.float32),
    )
    res = run_bass_kernel_spmd(nc, in_maps, core_ids=list(range(N_CORES)),
                               **RUN_KWARGS)
    LAST_RESULT = res
    full = _assemble([r["out"] for r in res.results], rows)
    return full.reshape(n, t, V)

